# revision 23
# baseline (speedup 1.0000x reference)
"""Trainium2 Bass kernel for the 2-layer tanh RNN (nn_RNN_47949014892907).

Sharding: data-parallel over batch (128 -> 16 per core, 8 cores), zero
cross-core communication. Per core:
  phase A: gather embedding rows (dma_gather transpose -> xT layout) and
           compute X0 = wx0 @ x_t + b0 for all timesteps (parallel work)
  phase B: 128-step recurrence, all matmuls in [H,B] "transposed" layout:
           h0' = tanh(wh0 @ h0 + X0(t)); h1' = tanh(wx1 @ h0' + wh1 @ h1 + b1)
  phase C: logits = h1'(t) @ wy.T + wy_b with wy streamed in column blocks,
           activations (H1all) stationary on the PE.
All matmul operands bf16 (fp32 PSUM accumulation); biases applied in fp32.
"""

import sys
import os

sys.path.insert(0, "/opt/trn_rl_repo")

import numpy as np
import ml_dtypes
from contextlib import ExitStack

import concourse.bass as bass
import concourse.bacc as bacc
import concourse.mybir as mybir
import concourse.tile as tile
from concourse.bass_utils import run_bass_kernel_spmd

BF16 = ml_dtypes.bfloat16

# problem sizes (hardcoded per contract)
S = 128          # sequence length
B = 128          # global batch
NC = 8           # cores
NB = B // NC     # local batch = 16
H = 1024
E = 1024
V = 10000
KT = H // 128    # 8 contraction tiles
MT = H // 128    # 8 output tiles
TOK = S * NB     # 2048 local tokens
TT = TOK // 128  # 16 token tiles

F32 = mybir.dt.float32
BF = mybir.dt.bfloat16
AF = mybir.ActivationFunctionType
ADD = mybir.AluOpType.add

# logits vocab blocking: first block small so the logits phase starts on a
# ~1MB DMA right after the chain; rest are 2048-col blocks
VBLOCKS = [512, 2048, 2048, 2048, 2048, 1296]
VBMAX = 2048
assert sum(VBLOCKS) == V


def _vchunks(n):
    """split a column block into psum-sized chunks of <=512"""
    out, off = [], 0
    while off < n:
        w = min(512, n - off)
        out.append((off, w))
        off += w
    return out


def _emit(nc, tc, t_in, t_out):
    ctx = ExitStack()
    with ctx:
        idxw = t_in["idxw"].ap()
        embb = t_in["embb"].ap()
        wx0d = t_in["wx0T"].ap().rearrange("(k p) h -> p k h", p=128)
        wh0d = t_in["wh0T"].ap().rearrange("(k p) h -> p k h", p=128)
        wx1d = t_in["wx1T"].ap().rearrange("(k p) h -> p k h", p=128)
        wh1d = t_in["wh1T"].ap().rearrange("(k p) h -> p k h", p=128)
        wyd = t_in["wyT"].ap().rearrange("(k p) v -> p k v", p=128)
        b0d = t_in["b0T"].ap()
        b1d = t_in["b1T"].ap()
        wybd = t_in["wybR"].ap()
        hTid = t_in["hTi"].ap()
        log_o = t_out["logits_sh"].ap()
        fh_o = t_out["final_hT"].ap()

        # ---- long-lived pool: H1all + biases + h-state ----
        pool_long = ctx.enter_context(tc.tile_pool(name="long", bufs=1))
        H1all = pool_long.tile([128, KT, TOK], BF)
        b1_sb = pool_long.tile([128, KT], F32, tag="b1")
        h0_init = pool_long.tile([128, KT, NB], BF, tag="h0i")
        h1_init = pool_long.tile([128, KT, NB], BF, tag="h1i")
        fh_sb = pool_long.tile([128, 2, KT, NB], F32, tag="fh")
        nc.sync.dma_start(out=b1_sb[:], in_=b1d[:])
        nc.sync.dma_start(out=h0_init[:], in_=hTid[0])
        nc.sync.dma_start(out=h1_init[:], in_=hTid[1])

        # ---- pool spanning phases A+B (freed before logits) ----
        with tc.tile_pool(name="ab", bufs=1) as pool_ab:
            X0T = pool_ab.tile([128, MT, TOK], BF, tag="x0")
            wh0_sb = pool_ab.tile([128, KT, H], BF, tag="wh0")
            wx1_sb = pool_ab.tile([128, KT, H], BF, tag="wx1")
            wh1_sb = pool_ab.tile([128, KT, H], BF, tag="wh1")

            # ================= phase A: gather + X0 =================
            with (
                tc.tile_pool(name="ph_a", bufs=1) as pool_a,
                tc.tile_pool(name="ph_a_ps", bufs=4, space="PSUM") as psum_a,
            ):
                idx_sb = pool_a.tile([128, TOK // 16], mybir.dt.int16, tag="idx")
                nc.sync.dma_start(out=idx_sb[:], in_=idxw[:])
                GCHUNKS = [256, 256, 512, 512, 512]
                xT_chunks = []
                goff = 0
                for g, gw in enumerate(GCHUNKS):
                    xt = pool_a.tile([128, KT, gw], BF,
                                     name=f"xT{g}", tag=f"xT{g}")
                    nc.gpsimd.dma_gather(
                        out_ap=xt[:],
                        in_ap=embb[:],
                        idxs_ap=idx_sb[:, goff // 16:(goff + gw) // 16],
                        num_idxs=gw,
                        num_idxs_reg=gw,
                        elem_size=E,
                        transpose=True,
                    )
                    xT_chunks.append(xt)
                    goff += gw
                if "xT_dbg" in t_out:
                    nc.sync.dma_start(out=t_out["xT_dbg"].ap(),
                                      in_=xT_chunks[0][:])
                wx0_sb = pool_a.tile([128, KT, H], BF, tag="wx0")
                nc.sync.dma_start(out=wx0_sb[:], in_=wx0d[:])
                b0_sb = pool_a.tile([128, KT], F32, tag="b0")
                nc.sync.dma_start(out=b0_sb[:], in_=b0d[:])

                coff = 0
                for c, gw in enumerate(GCHUNKS):
                    for m in range(MT):
                        ps = psum_a.tile([128, 512], F32, tag="ps",
                                         name=f"x0ps_{c}_{m}")
                        for k in range(KT):
                            nc.tensor.matmul(
                                ps[:, :gw],
                                wx0_sb[:, k, m * 128:(m + 1) * 128],
                                xT_chunks[c][:, k, :],
                                start=(k == 0),
                                stop=(k == KT - 1),
                            )
                        # X0 = wx0 @ x + b0 (bias per partition = per hidden unit)
                        nc.scalar.activation(
                            X0T[:, m, coff:coff + gw],
                            ps[:, :gw],
                            AF.Identity,
                            bias=b0_sb[:, m:m + 1],
                        )
                    coff += gw

            # chain weights load behind the gather/wx0 DMAs (needed later)
            nc.sync.dma_start(out=wh0_sb[:], in_=wh0d[:])
            nc.sync.dma_start(out=wx1_sb[:], in_=wx1d[:])
            nc.sync.dma_start(out=wh1_sb[:], in_=wh1d[:])

            # ================= phase B: recurrence =================
            # pass 1: h0-chain only (wh0 per step); wx1 deferred + batched.
            with tc.tile_pool(name="ph_b_ps", bufs=2, space="PSUM") as psum_b:
                H0all = pool_ab.tile([128, KT, TOK], BF, tag="h0all")
                MG = [(0, 3), (3, 6), (6, 7), (7, 8)]
                h0_prev = h0_init
                for t in range(S):
                    ts = slice(t * NB, (t + 1) * NB)
                    for p, (ma, mb) in enumerate(MG):
                        nw = mb - ma
                        ps = psum_b.tile([128, 3, NB], F32, tag=f"ps{p}",
                                         name=f"psa{p}_{t}")
                        for mm in range(nw):
                            m = ma + mm
                            for k in range(KT):
                                nc.tensor.matmul(
                                    ps[:, mm, :],
                                    wh0_sb[:, k, m * 128:(m + 1) * 128],
                                    h0_prev[:, k, :],
                                    start=(k == 0),
                                    stop=(k == KT - 1),
                                )
                        mp = slice(ma, mb)
                        nc.vector.tensor_tensor(
                            ps[:, :nw, :], ps[:, :nw, :], X0T[:, mp, ts],
                            op=ADD,
                        )
                        nc.scalar.activation(
                            H0all[:, mp, ts], ps[:, :nw, :], AF.Tanh,
                        )
                    h0_prev = H0all[:, :, ts]

                # Z1 = wx1 @ H0all + b1, batched over all tokens
                Z1T = pool_ab.tile([128, MT, TOK], BF, tag="x0")
                for c in range(TOK // 512):
                    for m in range(MT):
                        psz = psum_b.tile([128, 512], F32, tag="ps0", name=f"psz_{m}_{c}")
                        for k in range(KT):
                            nc.tensor.matmul(
                                psz[:],
                                wx1_sb[:, k, m * 128:(m + 1) * 128],
                                H0all[:, k, c * 512:(c + 1) * 512],
                                start=(k == 0),
                                stop=(k == KT - 1),
                            )
                        nc.scalar.activation(
                            Z1T[:, m, c * 512:(c + 1) * 512],
                            psz[:],
                            AF.Identity,
                            bias=b1_sb[:, m:m + 1],
                        )

                # pass 2: h1-chain (wh1 per step)
                h1_prev = h1_init
                for t in range(S):
                    ts = slice(t * NB, (t + 1) * NB)
                    for p, (ma, mb) in enumerate(MG):
                        nw = mb - ma
                        ps = psum_b.tile([128, 3, NB], F32, tag=f"ps{p}",
                                         name=f"psb{p}_{t}")
                        for mm in range(nw):
                            m = ma + mm
                            for k in range(KT):
                                nc.tensor.matmul(
                                    ps[:, mm, :],
                                    wh1_sb[:, k, m * 128:(m + 1) * 128],
                                    h1_prev[:, k, :],
                                    start=(k == 0),
                                    stop=(k == KT - 1),
                                )
                        mp = slice(ma, mb)
                        nc.vector.tensor_tensor(
                            ps[:, :nw, :], ps[:, :nw, :], Z1T[:, mp, ts],
                            op=ADD,
                        )
                        nc.scalar.activation(
                            H1all[:, mp, ts], ps[:, :nw, :], AF.Tanh,
                        )
                    h1_prev = H1all[:, :, ts]

                # final hidden state out (fp32)
                nc.vector.tensor_copy(
                    fh_sb[:, 0], H0all[:, :, (S - 1) * NB:S * NB]
                )
                nc.vector.tensor_copy(
                    fh_sb[:, 1], H1all[:, :, (S - 1) * NB:S * NB]
                )
                nc.sync.dma_start(out=fh_o[:], in_=fh_sb[:])
                if "X0T_dbg" in t_out:
                    nc.sync.dma_start(out=t_out["X0T_dbg"].ap(), in_=X0T[:])
                if "H1_dbg" in t_out:
                    nc.sync.dma_start(out=t_out["H1_dbg"].ap(), in_=H1all[:])

        # ================= phase C: logits =================
        with (
            tc.tile_pool(name="ph_c_wy", bufs=2) as pool_wy,
            tc.tile_pool(name="ph_c_wb", bufs=2) as pool_wb,
            tc.tile_pool(name="ph_c_st", bufs=3) as pool_st,
            tc.tile_pool(name="ph_c_ps", bufs=2, space="PSUM") as psum_c,
        ):
            voff = 0
            for vb, vw in enumerate(VBLOCKS):
                wy_sb = pool_wy.tile([128, KT, VBMAX], BF, tag="wy",
                                     name=f"wy_{vb}")
                nc.sync.dma_start(
                    out=wy_sb[:, :, :vw], in_=wyd[:, :, voff:voff + vw]
                )
                wyb_sb = pool_wb.tile([128, VBMAX], F32, tag="wybs",
                                      name=f"wyb_{vb}")
                nc.sync.dma_start(
                    out=wyb_sb[:, :vw], in_=wybd[:, voff:voff + vw]
                )
                chunks = _vchunks(vw)
                for tt in range(TT):
                    pss = [
                        psum_c.tile([128, 512], F32, tag=f"lps{i}",
                                    name=f"lps{i}_{vb}_{tt}")
                        for i in range(len(chunks))
                    ]
                    for k in range(KT):
                        lhsT = H1all[:, k, tt * 128:(tt + 1) * 128]
                        for i, (co, cw) in enumerate(chunks):
                            nc.tensor.matmul(
                                pss[i][:, :cw],
                                lhsT,
                                wy_sb[:, k, co:co + cw],
                                start=(k == 0),
                                stop=(k == KT - 1),
                            )
                    stage = pool_st.tile([128, VBMAX], F32, tag="stage")
                    for i, (co, cw) in enumerate(chunks):
                        nc.vector.tensor_tensor(
                            stage[:, co:co + cw],
                            pss[i][:, :cw],
                            wyb_sb[:, co:co + cw],
                            op=ADD,
                        )
                    nc.sync.dma_start(
                        out=log_o[tt * 128:(tt + 1) * 128, voff:voff + vw],
                        in_=stage[:, :vw],
                    )
                voff += vw


_CACHE = {}


def _build():
    if "nc" in _CACHE:
        return _CACHE["nc"], _CACHE["t_in"], _CACHE["t_out"]
    nc = bacc.Bacc(
        "TRN2", target_bir_lowering=False, debug=False, num_devices=NC
    )
    t_in = {}
    t_out = {}

    def din(name, shape, dt):
        t_in[name] = nc.dram_tensor(name, shape, dt, kind="ExternalInput")

    def dout(name, shape, dt):
        t_out[name] = nc.dram_tensor(name, shape, dt, kind="ExternalOutput")

    din("idxw", [128, TOK // 16], mybir.dt.int16)
    din("embb", [V, E], BF)
    din("wx0T", [E, H], BF)
    din("wh0T", [H, H], BF)
    din("wx1T", [H, H], BF)
    din("wh1T", [H, H], BF)
    din("wyT", [H, V], BF)
    din("b0T", [128, KT], F32)
    din("b1T", [128, KT], F32)
    din("wybR", [128, V], F32)
    din("hTi", [2, 128, KT, NB], BF)
    dout("logits_sh", [TOK, V], F32)
    if os.environ.get("KERNEL_DEBUG"):
        dout("xT_dbg", [128, KT, 512], BF)
        dout("X0T_dbg", [128, MT, TOK], BF)
        dout("H1_dbg", [128, KT, TOK], BF)
    dout("final_hT", [128, 2, KT, NB], F32)

    with tile.TileContext(nc) as tc:
        _emit(nc, tc, t_in, t_out)
    nc.compile()
    _CACHE.update(nc=nc, t_in=t_in, t_out=t_out)
    return nc, t_in, t_out


def _prep_in_maps(inputs):
    idx = np.asarray(inputs["inputs"]).astype(np.int64)   # [S, B]
    hidden = np.asarray(inputs["hidden"], dtype=np.float32)  # [2, B, H]
    emb = np.asarray(inputs["emb"], dtype=np.float32)
    wx0 = np.asarray(inputs["wx0"], dtype=np.float32)
    wx1 = np.asarray(inputs["wx_rest"], dtype=np.float32)[0]
    wh0 = np.asarray(inputs["wh_w"], dtype=np.float32)[0]
    wh1 = np.asarray(inputs["wh_w"], dtype=np.float32)[1]
    whb = np.asarray(inputs["wh_b"], dtype=np.float32)
    wy = np.asarray(inputs["wy_w"], dtype=np.float32)
    wyb = np.asarray(inputs["wy_b"], dtype=np.float32)

    shared = {
        "embb": np.ascontiguousarray(emb.astype(BF16)),
        "wx0T": np.ascontiguousarray(wx0.T).astype(BF16),
        "wh0T": np.ascontiguousarray(wh0.T).astype(BF16),
        "wx1T": np.ascontiguousarray(wx1.T).astype(BF16),
        "wh1T": np.ascontiguousarray(wh1.T).astype(BF16),
        "wyT": np.ascontiguousarray(wy.T).astype(BF16),
        "b0T": np.ascontiguousarray(whb[0].reshape(KT, 128).T),
        "b1T": np.ascontiguousarray(whb[1].reshape(KT, 128).T),
        "wybR": np.ascontiguousarray(np.broadcast_to(wyb, (128, V))),
    }
    in_maps = []
    for c in range(NC):
        sl = slice(c * NB, (c + 1) * NB)
        idx_flat = np.ascontiguousarray(idx[:, sl]).reshape(TOK)  # t-major
        idxw = np.tile(
            np.ascontiguousarray(idx_flat.reshape(TOK // 16, 16).T), (8, 1)
        ).astype(np.int16)
        # h[l].T in k-tile layout: [128, KT, NB]
        hT = np.ascontiguousarray(
            hidden[:, sl, :].transpose(0, 2, 1)  # [2, H, NB]
        ).reshape(2, KT, 128, NB).transpose(0, 2, 1, 3)
        in_maps.append(
            dict(shared, idxw=idxw, hTi=np.ascontiguousarray(hT).astype(BF16))
        )
    return in_maps


def _assemble(results):
    logits = np.empty((S, B, V), dtype=np.float32)
    final_h = np.empty((2, B, H), dtype=np.float32)
    for c in range(NC):
        sl = slice(c * NB, (c + 1) * NB)
        lg = results[c]["logits_sh"].reshape(S, NB, V)
        logits[:, sl, :] = lg
        fh = results[c]["final_hT"].reshape(128, 2, KT, NB)
        # fh[p, l, k, b] -> final_h[l, b, k*128+p]
        final_h[:, sl, :] = fh.transpose(1, 3, 2, 0).reshape(2, NB, H)
    return logits, final_h


LAST_PROFILE = {}


def kernel(**inputs):
    nc, t_in, t_out = _build()
    in_maps = _prep_in_maps(inputs)
    res = run_bass_kernel_spmd(nc, in_maps, list(range(NC)), trace=False)
    LAST_PROFILE["exec_time_ns"] = res.exec_time_ns
    return _assemble(res.results)


# revision 24
# speedup vs baseline: 1.0746x; 1.0746x over previous
"""Trainium2 Bass kernel for the 2-layer tanh RNN (nn_RNN_47949014892907).

Sharding: data-parallel over batch (128 -> 16 per core, 8 cores), zero
cross-core communication. Per core:
  phase A: gather embedding rows (dma_gather transpose -> xT layout) and
           compute X0 = wx0 @ x_t + b0 for all timesteps (parallel work)
  phase B: 128-step recurrence, all matmuls in [H,B] "transposed" layout:
           h0' = tanh(wh0 @ h0 + X0(t)); h1' = tanh(wx1 @ h0' + wh1 @ h1 + b1)
  phase C: logits = h1'(t) @ wy.T + wy_b with wy streamed in column blocks,
           activations (H1all) stationary on the PE.
All matmul operands bf16 (fp32 PSUM accumulation); biases applied in fp32.
"""

import sys
import os

sys.path.insert(0, "/opt/trn_rl_repo")

import numpy as np
import ml_dtypes
from contextlib import ExitStack

import concourse.bass as bass
import concourse.bacc as bacc
import concourse.mybir as mybir
import concourse.tile as tile
from concourse.bass_utils import run_bass_kernel_spmd

BF16 = ml_dtypes.bfloat16

# problem sizes (hardcoded per contract)
S = 128          # sequence length
B = 128          # global batch
NC = 8           # cores
NB = B // NC     # local batch = 16
H = 1024
E = 1024
V = 10000
KT = H // 128    # 8 contraction tiles
MT = H // 128    # 8 output tiles
TOK = S * NB     # 2048 local tokens
TT = TOK // 128  # 16 token tiles

F32 = mybir.dt.float32
BF = mybir.dt.bfloat16
AF = mybir.ActivationFunctionType
ADD = mybir.AluOpType.add

# logits vocab blocking: first block small so the logits phase starts on a
# ~1MB DMA right after the chain; rest are 2048-col blocks
VBLOCKS = [512, 2048, 2048, 2048, 2048, 1296]
VBMAX = 2048
assert sum(VBLOCKS) == V


def _vchunks(n):
    """split a column block into psum-sized chunks of <=512"""
    out, off = [], 0
    while off < n:
        w = min(512, n - off)
        out.append((off, w))
        off += w
    return out


def _emit(nc, tc, t_in, t_out):
    ctx = ExitStack()
    with ctx:
        idxw = t_in["idxw"].ap()
        embb = t_in["embb"].ap()
        wx0d = t_in["wx0T"].ap().rearrange("(k p) h -> p k h", p=128)
        wh0d = t_in["wh0T"].ap().rearrange("(k p) h -> p k h", p=128)
        wx1d = t_in["wx1T"].ap().rearrange("(k p) h -> p k h", p=128)
        wh1d = t_in["wh1T"].ap().rearrange("(k p) h -> p k h", p=128)
        wyd = t_in["wyT"].ap().rearrange("(k p) v -> p k v", p=128)
        b0d = t_in["b0T"].ap()
        b1d = t_in["b1T"].ap()
        wybd = t_in["wybR"].ap()
        hTid = t_in["hTi"].ap()
        log_o = t_out["logits_sh"].ap()
        fh_o = t_out["final_hT"].ap()

        # ---- long-lived pool: H1all + biases + h-state ----
        pool_long = ctx.enter_context(tc.tile_pool(name="long", bufs=1))
        H1all = pool_long.tile([128, KT, TOK], BF)
        b1_sb = pool_long.tile([128, KT], F32, tag="b1")
        h0_init = pool_long.tile([128, KT, NB], BF, tag="h0i")
        h1_init = pool_long.tile([128, KT, NB], BF, tag="h1i")
        fh_sb = pool_long.tile([128, 2, KT, NB], F32, tag="fh")
        nc.sync.dma_start(out=b1_sb[:], in_=b1d[:])
        nc.sync.dma_start(out=h0_init[:], in_=hTid[0])
        nc.sync.dma_start(out=h1_init[:], in_=hTid[1])

        # ---- pool spanning phases A+B (freed before logits) ----
        with tc.tile_pool(name="ab", bufs=1) as pool_ab:
            X0T = pool_ab.tile([128, MT, TOK], BF, tag="x0")
            wh0_sb = pool_ab.tile([128, KT, H], BF, tag="wh0")
            wx1_sb = pool_ab.tile([128, KT, H], BF, tag="wx1")
            wh1_sb = pool_ab.tile([128, KT, H], BF, tag="wh1")

            # ================= phase A: gather + X0 =================
            with (
                tc.tile_pool(name="ph_a", bufs=1) as pool_a,
                tc.tile_pool(name="ph_a_ps", bufs=4, space="PSUM") as psum_a,
            ):
                idx_sb = pool_a.tile([128, TOK // 16], mybir.dt.int16, tag="idx")
                nc.sync.dma_start(out=idx_sb[:], in_=idxw[:])
                GCHUNKS = [256, 256, 512, 512, 512]
                xT_chunks = []
                goff = 0
                for g, gw in enumerate(GCHUNKS):
                    xt = pool_a.tile([128, KT, gw], BF,
                                     name=f"xT{g}", tag=f"xT{g}")
                    nc.gpsimd.dma_gather(
                        out_ap=xt[:],
                        in_ap=embb[:],
                        idxs_ap=idx_sb[:, goff // 16:(goff + gw) // 16],
                        num_idxs=gw,
                        num_idxs_reg=gw,
                        elem_size=E,
                        transpose=True,
                    )
                    xT_chunks.append(xt)
                    goff += gw
                if "xT_dbg" in t_out:
                    nc.sync.dma_start(out=t_out["xT_dbg"].ap(),
                                      in_=xT_chunks[0][:])
                wx0_sb = pool_a.tile([128, KT, H], BF, tag="wx0")
                nc.sync.dma_start(out=wx0_sb[:], in_=wx0d[:])
                b0_sb = pool_a.tile([128, KT], F32, tag="b0")
                nc.sync.dma_start(out=b0_sb[:], in_=b0d[:])

                coff = 0
                for c, gw in enumerate(GCHUNKS):
                    for m in range(MT):
                        ps = psum_a.tile([128, 512], F32, tag="ps",
                                         name=f"x0ps_{c}_{m}")
                        for k in range(KT):
                            nc.tensor.matmul(
                                ps[:, :gw],
                                wx0_sb[:, k, m * 128:(m + 1) * 128],
                                xT_chunks[c][:, k, :],
                                start=(k == 0),
                                stop=(k == KT - 1),
                            )
                        # X0 = wx0 @ x + b0 (bias per partition = per hidden unit)
                        nc.scalar.activation(
                            X0T[:, m, coff:coff + gw],
                            ps[:, :gw],
                            AF.Identity,
                            bias=b0_sb[:, m:m + 1],
                        )
                    coff += gw

            # chain weights load behind the gather/wx0 DMAs (needed later)
            nc.sync.dma_start(out=wh0_sb[:], in_=wh0d[:])
            nc.sync.dma_start(out=wx1_sb[:], in_=wx1d[:])
            nc.sync.dma_start(out=wh1_sb[:], in_=wh1d[:])

            # ================= phase B: recurrence =================
            # pass 1: h0-chain only (wh0 per step); wx1 deferred + batched.
            with tc.tile_pool(name="ph_b_ps", bufs=2, space="PSUM") as psum_b:
                H0all = pool_ab.tile([128, KT, TOK], BF, tag="h0all")
                h0_prev = h0_init
                for t in range(S):
                    ts = slice(t * NB, (t + 1) * NB)
                    for p in range(4):
                        ps = psum_b.tile([128, 2, NB], F32, tag=f"ps{p}",
                                         name=f"psa{p}_{t}")
                        for mm in range(2):
                            m = p * 2 + mm
                            for k in range(KT):
                                nc.tensor.matmul(
                                    ps[:, mm, :],
                                    wh0_sb[:, k, m * 128:(m + 1) * 128],
                                    h0_prev[:, k, :],
                                    start=(k == 0),
                                    stop=(k == KT - 1),
                                )
                        mp = slice(p * 2, p * 2 + 2)
                        nc.vector.tensor_tensor(
                            ps[:], ps[:], X0T[:, mp, ts], op=ADD,
                        )
                        nc.scalar.activation(
                            H0all[:, mp, ts], ps[:], AF.Tanh,
                        )
                    h0_prev = H0all[:, :, ts]

                # Z1 = wx1 @ H0all + b1, batched over all tokens
                Z1T = pool_ab.tile([128, MT, TOK], BF, tag="x0")
                for c in range(TOK // 512):
                    for m in range(MT):
                        psz = psum_b.tile([128, 512], F32, tag="ps0", name=f"psz_{m}_{c}")
                        for k in range(KT):
                            nc.tensor.matmul(
                                psz[:],
                                wx1_sb[:, k, m * 128:(m + 1) * 128],
                                H0all[:, k, c * 512:(c + 1) * 512],
                                start=(k == 0),
                                stop=(k == KT - 1),
                            )
                        nc.scalar.activation(
                            Z1T[:, m, c * 512:(c + 1) * 512],
                            psz[:],
                            AF.Identity,
                            bias=b1_sb[:, m:m + 1],
                        )

                # pass 2: h1-chain (wh1 per step)
                h1_prev = h1_init
                for t in range(S):
                    ts = slice(t * NB, (t + 1) * NB)
                    for p in range(4):
                        ps = psum_b.tile([128, 2, NB], F32, tag=f"ps{p}",
                                         name=f"psb{p}_{t}")
                        for mm in range(2):
                            m = p * 2 + mm
                            for k in range(KT):
                                nc.tensor.matmul(
                                    ps[:, mm, :],
                                    wh1_sb[:, k, m * 128:(m + 1) * 128],
                                    h1_prev[:, k, :],
                                    start=(k == 0),
                                    stop=(k == KT - 1),
                                )
                        mp = slice(p * 2, p * 2 + 2)
                        nc.vector.tensor_tensor(
                            ps[:], ps[:], Z1T[:, mp, ts], op=ADD,
                        )
                        nc.scalar.activation(
                            H1all[:, mp, ts], ps[:], AF.Tanh,
                        )
                    h1_prev = H1all[:, :, ts]

                # final hidden state out (fp32)
                nc.vector.tensor_copy(
                    fh_sb[:, 0], H0all[:, :, (S - 1) * NB:S * NB]
                )
                nc.vector.tensor_copy(
                    fh_sb[:, 1], H1all[:, :, (S - 1) * NB:S * NB]
                )
                nc.sync.dma_start(out=fh_o[:], in_=fh_sb[:])
                if "X0T_dbg" in t_out:
                    nc.sync.dma_start(out=t_out["X0T_dbg"].ap(), in_=X0T[:])
                if "H1_dbg" in t_out:
                    nc.sync.dma_start(out=t_out["H1_dbg"].ap(), in_=H1all[:])

        # ================= phase C: logits =================
        with (
            tc.tile_pool(name="ph_c_wy", bufs=2) as pool_wy,
            tc.tile_pool(name="ph_c_wb", bufs=2) as pool_wb,
            tc.tile_pool(name="ph_c_st", bufs=3) as pool_st,
            tc.tile_pool(name="ph_c_ps", bufs=2, space="PSUM") as psum_c,
        ):
            voff = 0
            for vb, vw in enumerate(VBLOCKS):
                wy_sb = pool_wy.tile([128, KT, VBMAX], BF, tag="wy",
                                     name=f"wy_{vb}")
                nc.sync.dma_start(
                    out=wy_sb[:, :, :vw], in_=wyd[:, :, voff:voff + vw]
                )
                wyb_sb = pool_wb.tile([128, VBMAX], F32, tag="wybs",
                                      name=f"wyb_{vb}")
                nc.sync.dma_start(
                    out=wyb_sb[:, :vw], in_=wybd[:, voff:voff + vw]
                )
                chunks = _vchunks(vw)
                for tt in range(TT):
                    pss = [
                        psum_c.tile([128, 512], F32, tag=f"lps{i}",
                                    name=f"lps{i}_{vb}_{tt}")
                        for i in range(len(chunks))
                    ]
                    for k in range(KT):
                        lhsT = H1all[:, k, tt * 128:(tt + 1) * 128]
                        for i, (co, cw) in enumerate(chunks):
                            nc.tensor.matmul(
                                pss[i][:, :cw],
                                lhsT,
                                wy_sb[:, k, co:co + cw],
                                start=(k == 0),
                                stop=(k == KT - 1),
                            )
                    stage = pool_st.tile([128, VBMAX], F32, tag="stage")
                    for i, (co, cw) in enumerate(chunks):
                        nc.vector.tensor_tensor(
                            stage[:, co:co + cw],
                            pss[i][:, :cw],
                            wyb_sb[:, co:co + cw],
                            op=ADD,
                        )
                    nc.sync.dma_start(
                        out=log_o[tt * 128:(tt + 1) * 128, voff:voff + vw],
                        in_=stage[:, :vw],
                    )
                voff += vw


_CACHE = {}


def _build():
    if "nc" in _CACHE:
        return _CACHE["nc"], _CACHE["t_in"], _CACHE["t_out"]
    nc = bacc.Bacc(
        "TRN2", target_bir_lowering=False, debug=False, num_devices=NC
    )
    t_in = {}
    t_out = {}

    def din(name, shape, dt):
        t_in[name] = nc.dram_tensor(name, shape, dt, kind="ExternalInput")

    def dout(name, shape, dt):
        t_out[name] = nc.dram_tensor(name, shape, dt, kind="ExternalOutput")

    din("idxw", [128, TOK // 16], mybir.dt.int16)
    din("embb", [V, E], BF)
    din("wx0T", [E, H], BF)
    din("wh0T", [H, H], BF)
    din("wx1T", [H, H], BF)
    din("wh1T", [H, H], BF)
    din("wyT", [H, V], BF)
    din("b0T", [128, KT], F32)
    din("b1T", [128, KT], F32)
    din("wybR", [128, V], F32)
    din("hTi", [2, 128, KT, NB], BF)
    dout("logits_sh", [TOK, V], F32)
    if os.environ.get("KERNEL_DEBUG"):
        dout("xT_dbg", [128, KT, 512], BF)
        dout("X0T_dbg", [128, MT, TOK], BF)
        dout("H1_dbg", [128, KT, TOK], BF)
    dout("final_hT", [128, 2, KT, NB], F32)

    with tile.TileContext(nc) as tc:
        _emit(nc, tc, t_in, t_out)
    nc.compile()
    _CACHE.update(nc=nc, t_in=t_in, t_out=t_out)
    return nc, t_in, t_out


def _prep_in_maps(inputs):
    idx = np.asarray(inputs["inputs"]).astype(np.int64)   # [S, B]
    hidden = np.asarray(inputs["hidden"], dtype=np.float32)  # [2, B, H]
    emb = np.asarray(inputs["emb"], dtype=np.float32)
    wx0 = np.asarray(inputs["wx0"], dtype=np.float32)
    wx1 = np.asarray(inputs["wx_rest"], dtype=np.float32)[0]
    wh0 = np.asarray(inputs["wh_w"], dtype=np.float32)[0]
    wh1 = np.asarray(inputs["wh_w"], dtype=np.float32)[1]
    whb = np.asarray(inputs["wh_b"], dtype=np.float32)
    wy = np.asarray(inputs["wy_w"], dtype=np.float32)
    wyb = np.asarray(inputs["wy_b"], dtype=np.float32)

    shared = {
        "embb": np.ascontiguousarray(emb.astype(BF16)),
        "wx0T": np.ascontiguousarray(wx0.T).astype(BF16),
        "wh0T": np.ascontiguousarray(wh0.T).astype(BF16),
        "wx1T": np.ascontiguousarray(wx1.T).astype(BF16),
        "wh1T": np.ascontiguousarray(wh1.T).astype(BF16),
        "wyT": np.ascontiguousarray(wy.T).astype(BF16),
        "b0T": np.ascontiguousarray(whb[0].reshape(KT, 128).T),
        "b1T": np.ascontiguousarray(whb[1].reshape(KT, 128).T),
        "wybR": np.ascontiguousarray(np.broadcast_to(wyb, (128, V))),
    }
    in_maps = []
    for c in range(NC):
        sl = slice(c * NB, (c + 1) * NB)
        idx_flat = np.ascontiguousarray(idx[:, sl]).reshape(TOK)  # t-major
        idxw = np.tile(
            np.ascontiguousarray(idx_flat.reshape(TOK // 16, 16).T), (8, 1)
        ).astype(np.int16)
        # h[l].T in k-tile layout: [128, KT, NB]
        hT = np.ascontiguousarray(
            hidden[:, sl, :].transpose(0, 2, 1)  # [2, H, NB]
        ).reshape(2, KT, 128, NB).transpose(0, 2, 1, 3)
        in_maps.append(
            dict(shared, idxw=idxw, hTi=np.ascontiguousarray(hT).astype(BF16))
        )
    return in_maps


def _assemble(results):
    logits = np.empty((S, B, V), dtype=np.float32)
    final_h = np.empty((2, B, H), dtype=np.float32)
    for c in range(NC):
        sl = slice(c * NB, (c + 1) * NB)
        lg = results[c]["logits_sh"].reshape(S, NB, V)
        logits[:, sl, :] = lg
        fh = results[c]["final_hT"].reshape(128, 2, KT, NB)
        # fh[p, l, k, b] -> final_h[l, b, k*128+p]
        final_h[:, sl, :] = fh.transpose(1, 3, 2, 0).reshape(2, NB, H)
    return logits, final_h


LAST_PROFILE = {}


def kernel(**inputs):
    nc, t_in, t_out = _build()
    in_maps = _prep_in_maps(inputs)
    res = run_bass_kernel_spmd(nc, in_maps, list(range(NC)), trace=False)
    LAST_PROFILE["exec_time_ns"] = res.exec_time_ns
    return _assemble(res.results)
